# revision 20
# baseline (speedup 1.0000x reference)
"""Trainium2 Bass kernel for nn_MultiHeadAttention (B=2, S=2048, DM=1024, H=8).

Sharding: data-parallel on batch x tensor-parallel on heads.
Core c in 0..7 handles batch b = c//4 and heads {2*(c%4), 2*(c%4)+1}.
Each core computes its two heads' full attention and the partial
out-projection (a 1024x2048 partial sum in fp16); the host adds the 4
partials per batch and transposes back to (S, DOUT).

v3 structure (why):
  - the kernel is PE-streaming-bound (~140us of bf16 matmul columns);
    everything else is scheduled to keep the PE queue from ever waiting.
  - x ring of 14 bufs; w/x interleaved in need order on 3 DMA queues.
  - mask is bf16 (DVE mask-multiply stays in 2x mode; fp8 operands
    halve DVE throughput) but shipped as 32 half-column tiles
    [128,1024] through a 20-buf ring: only half0's 4.2MB must land
    before attention starts; half1's tiles stream during half0's
    attention when HBM is otherwise idle.
  - scalar queue carries only ungated DMAs (its ACT evacuations can
    never sit behind a ring-gated DMA semaphore); gated x-t2 and
    half1-mask DMAs live on sync/gpsimd.
  - PSUM bias-evacuation on ACT (Identity + per-partition bias AP).
  - reciprocal on a [128,8]-repacked layout (DVE reciprocal cost is
    its free-dim size: [1,512] would be ~4us, [128,8] is ~0.3us).
  - out-projection of half0 is software-pipelined one tile per oc
    into half1-h0's attention loop so its PE/DVE work hides under the
    attention stream instead of serializing after it; half1's
    out-projection forms the tail with alternating DVE/ACT evacs.
  - outT in fp16 (halves output DMA; host sums partials in fp32).
PSUM budget (8 banks): tag acc 3 + tag s 2x[128,1024] = 4 + facc 1.
Projections therefore accumulate 2 query-tiles at a time (2 groups).
"""

import sys

sys.path.insert(0, "/opt/trn_rl_repo")

import numpy as np
import ml_dtypes

import concourse.bass as bass
import concourse.tile as tile
from concourse import bacc, mybir
from concourse.bass import ts, ds
from concourse.bass_utils import run_bass_kernel_spmd

BF16 = mybir.dt.bfloat16
F32 = mybir.dt.float32
F16 = mybir.dt.float16
Exp = mybir.ActivationFunctionType.Exp
Ident = mybir.ActivationFunctionType.Identity

B, S, DM, H, DOUT = 2, 2048, 1024, 8, 1024
D = DM // H            # 128 head dim
NH = 2                 # heads per core
KC = DM // 128         # 8 contraction chunks for projections
OC = S // 128          # 16 key chunks
NT = 512               # PSUM-bank-sized free tile (fp32)
IT = S // NT           # 4 query tiles
SCALE = float(1.0 / np.sqrt(np.float32(D)))


def build():
    nc = bacc.Bacc(None, target_bir_lowering=False)

    xT = nc.dram_tensor("xT", [3, DM, S], BF16, kind="ExternalInput")
    maskT = nc.dram_tensor("maskT", [S, S], BF16, kind="ExternalInput")
    w_qkv = nc.dram_tensor("w_qkv", [128, 3, KC, NH, D], BF16, kind="ExternalInput")
    b_qkv = nc.dram_tensor("b_qkv", [128, 3, NH], F32, kind="ExternalInput")
    wo = nc.dram_tensor("wo", [D, NH, DOUT], BF16, kind="ExternalInput")
    ident = nc.dram_tensor("ident", [128, 128], BF16, kind="ExternalInput")
    bo = nc.dram_tensor("bo", [128, DOUT // 128], F32, kind="ExternalInput")
    outT = nc.dram_tensor("outT", [DOUT, S], F16, kind="ExternalOutput")

    q_sg = [nc.sync, nc.gpsimd]
    q_sgs = [nc.sync, nc.gpsimd, nc.scalar]

    with tile.TileContext(nc) as tc:
        with (
            tc.tile_pool(name="const", bufs=1) as constp,
            tc.tile_pool(name="xin", bufs=16) as xp,
            tc.tile_pool(name="mask", bufs=20) as mp,
            tc.tile_pool(name="ptile", bufs=4) as pp,
            tc.tile_pool(name="rb", bufs=2) as rbp,
            tc.tile_pool(name="fout", bufs=4) as fop,
            tc.tile_pool(name="psum", bufs=3, space="PSUM") as psp,
            tc.tile_pool(name="dram", bufs=2, space="DRAM") as dramp,
        ):
            # ---- small constants (scalar queue; ungated, tiny) ----
            b_sb = constp.tile([128, 3, NH], F32)
            nc.scalar.dma_start(out=b_sb, in_=b_qkv[:])
            bo_sb = constp.tile([128, DOUT // 128], F32)
            nc.scalar.dma_start(out=bo_sb, in_=bo[:])
            ident_sb = constp.tile([128, 128], BF16)
            nc.scalar.dma_start(out=ident_sb, in_=ident[:])
            ones_col = constp.tile([128, 1], BF16)
            nc.vector.memset(ones_col, 1.0)

            # ---- weights + x tiles interleaved in need order ----
            w_sb = constp.tile([128, 3, KC, NH, D], BF16)
            wo_sb = constp.tile([D, NH, DOUT], BF16)
            xts_all = []
            for t in range(3):
                row = []
                for k in range(KC):
                    qi = t * KC + k
                    if t < 2:
                        q_w = q_sgs[(qi + 1) % 3]
                        q_x = q_sgs[qi % 3]
                    else:
                        # ring-gated x-t2: sync/gpsimd only; its (ungated)
                        # weights go early on scalar.
                        q_w = nc.scalar
                        q_x = q_sg[qi % 2]
                    q_w.dma_start(out=w_sb[:, t, k, :, :], in_=w_qkv[:, t, k, :, :])
                    xt = xp.tile([128, S], BF16, tag="x", name=f"x{t}_{k}")
                    q_x.dma_start(out=xt, in_=xT[t, k * 128 : (k + 1) * 128, :])
                    row.append(xt)
                xts_all.append(row)

            # mask half-column tiles [128,1024] keyed (half, oc). half0's 16
            # are queued here behind x-t2 (sync/gpsimd, few on scalar);
            # half1's are allocated+issued lazily inside half0-h1's
            # attention loop so they enter the in-order queues after the
            # half0 reciprocal-chain DMAs and stream while HBM is idle.
            mask_t = {}

            def mask_fetch(half, oc, qm):
                mt = mp.tile([128, 2 * NT], BF16, tag="m", name=f"m{half}_{oc}")
                qm.dma_start(
                    out=mt,
                    in_=maskT[
                        oc * 128 : (oc + 1) * 128,
                        half * 2 * NT : (half + 1) * 2 * NT,
                    ],
                )
                mask_t[(half, oc)] = mt

            for oc in range(OC):
                mask_fetch(0, oc, q_sg[oc % 2])
            nc.scalar.dma_start(out=wo_sb, in_=wo[:])

            # ---- Q/K projections: qk_sb[d, t, h, s] (2 query-tile groups,
            #      3-buf PSUM acc ring); bias added on ACT evac ----
            qk_sb = constp.tile([128, 2, NH, S], BF16)
            for t in range(2):
                xts = xts_all[t]
                for h in range(NH):
                    for g in range(2):
                        acc = [
                            psp.tile([128, NT], F32, tag="acc", name=f"acc{g}{i}")
                            for i in range(2)
                        ]
                        for k in range(KC):
                            for i in range(2):
                                nc.tensor.matmul(
                                    acc[i],
                                    w_sb[:, t, k, h, :],
                                    xts[k][:, ts(2 * g + i, NT)],
                                    start=(k == 0),
                                    stop=(k == KC - 1),
                                )
                        for i in range(2):
                            nc.scalar.activation(
                                out=qk_sb[:, t, h, ts(2 * g + i, NT)],
                                in_=acc[i],
                                func=Ident,
                                bias=b_sb[:, t, h : h + 1],
                            )

            # ---- V projection via VpT + PE transpose: vp_sb[s%128, oc, h, d] ----
            vpt_sb = constp.tile([128, NH, S], BF16)  # [d, h, s] transient
            vp_sb = constp.tile([128, OC, NH, D], BF16)
            xts = xts_all[2]
            for h in range(NH):
                for g in range(2):
                    acc = [
                        psp.tile([128, NT], F32, tag="acc", name=f"vacc{g}{i}")
                        for i in range(2)
                    ]
                    for k in range(KC):
                        for i in range(2):
                            nc.tensor.matmul(
                                acc[i],
                                w_sb[:, 2, k, h, :],
                                xts[k][:, ts(2 * g + i, NT)],
                                start=(k == 0),
                                stop=(k == KC - 1),
                            )
                    for i in range(2):
                        nc.scalar.activation(
                            out=vpt_sb[:, h, ts(2 * g + i, NT)],
                            in_=acc[i],
                            func=Ident,
                            bias=b_sb[:, 2, h : h + 1],
                        )
                for oc in range(OC):
                    tps = psp.tile([128, D], BF16, tag="s", bufs=2, name="tps")
                    nc.tensor.transpose(
                        tps, vpt_sb[:, h, ds(oc * 128, 128)], ident_sb
                    )
                    nc.vector.tensor_copy(vp_sb[:, oc, h, :], tps)

            outn_sb = constp.tile([128, NH, S], BF16)

            def outproj_item(it, dc, facc_tag, on_act, facc_bufs=1):
                """One out-projection tile: facc = sum_h wo_h^T outn_h."""
                facc = psp.tile(
                    [128, NT], F32, tag=facc_tag, bufs=facc_bufs, name="facc"
                )
                for h in range(NH):
                    nc.tensor.matmul(
                        facc,
                        wo_sb[:, h, ds(dc * 128, 128)],
                        outn_sb[:, h, ts(it, NT)],
                        start=(h == 0),
                        stop=(h == NH - 1),
                    )
                fsb = fop.tile([128, NT], F16, tag="f")
                if on_act:
                    nc.scalar.activation(
                        out=fsb, in_=facc, func=Ident,
                        bias=bo_sb[:, dc : dc + 1],
                    )
                else:
                    nc.vector.tensor_scalar_add(
                        out=fsb, in0=facc, scalar1=bo_sb[:, dc : dc + 1]
                    )
                q_sg[dc % 2].dma_start(
                    out=outT[dc * 128 : (dc + 1) * 128, ts(it, NT)], in_=fsb
                )

            def norm_chain(h, i0, rp, osb, jset, rowoff=32):
                """Rowsum rows (PSUM) -> DRAM -> [128,w] repack -> reciprocal
                -> DRAM -> per-j broadcast -> outn multiply. DVE reciprocal
                cost is its free-dim size, hence the repack."""
                w = 4 * len(jset)
                r2 = rbp.tile([33, NT], F32, tag="r2")
                for j in jset:
                    nc.vector.tensor_copy(
                        r2[rowoff * j : rowoff * j + 1, :],
                        rp[rowoff * j : rowoff * j + 1, :],
                    )
                rd = dramp.tile([len(jset), NT], F32, tag="rd")
                for n, j in enumerate(jset):
                    q_sg[j].dma_start(
                        out=rd[n : n + 1, :],
                        in_=r2[rowoff * j : rowoff * j + 1, :],
                    )
                rseg = rbp.tile([128, 8], F32, tag="rseg")
                nc.sync.dma_start(
                    out=rseg[:, :w],
                    in_=rd[:].rearrange("a b -> (a b)").rearrange(
                        "(p j) -> p j", p=128
                    ),
                )
                nc.vector.reciprocal(rseg[:, :w], rseg[:, :w])
                rd2 = dramp.tile([len(jset), NT], F32, tag="rd2")
                nc.gpsimd.dma_start(
                    out=rd2[:].rearrange("a b -> (a b)").rearrange(
                        "(p j) -> p j", p=128
                    ),
                    in_=rseg[:, :w],
                )
                for n, j in enumerate(jset):
                    rbc = rbp.tile([128, NT], F32, tag="rbc", bufs=2)
                    q_sg[j].dma_start(
                        out=rbc, in_=rd2[n : n + 1, :].to_broadcast([128, NT])
                    )
                    nc.vector.tensor_mul(
                        outn_sb[:, h, ds(i0 + j * NT, NT)],
                        osb[:, ts(j, NT)],
                        rbc,
                    )

            def attention(half, h, inject, prefetch=False):
                """One head's attention over 1024 queries; `inject` maps
                oc -> deferred out-projection items emitted there;
                `prefetch` issues the next half's mask DMAs one per oc."""
                i0 = half * 2 * NT
                outp = [
                    psp.tile([128, NT], F32, tag="acc", name=f"outp{j}")
                    for j in range(2)
                ]
                rp = psp.tile([128, NT], F32, tag="acc", name="rp")
                for oc in range(OC):
                    sps = psp.tile([128, 2 * NT], F32, tag="s", bufs=2, name="sps")
                    for j in range(2):
                        nc.tensor.matmul(
                            sps[:, ts(j, NT)],
                            qk_sb[:, 1, h, ds(oc * 128, 128)],
                            qk_sb[:, 0, h, ds(i0 + j * NT, NT)],
                            start=True,
                            stop=True,
                        )
                    p = pp.tile([128, 2 * NT], BF16, tag="p")
                    nc.scalar.activation(
                        out=p, in_=sps, func=Exp, bias=0.0, scale=SCALE
                    )
                    pm = pp.tile([128, 2 * NT], BF16, tag="pm")
                    nc.vector.tensor_mul(pm, p, mask_t[(half, oc)])
                    for j in range(2):
                        nc.tensor.matmul(
                            outp[j],
                            vp_sb[:, oc, h, :],
                            pm[:, ts(j, NT)],
                            start=(oc == 0),
                            stop=(oc == OC - 1),
                        )
                    for j in range(2):
                        nc.tensor.matmul(
                            rp[32 * j : 32 * j + 1, :],
                            ones_col,
                            pm[:, ts(j, NT)],
                            start=(oc == 0),
                            stop=(oc == OC - 1),
                        )
                    if prefetch:
                        mask_fetch(half + 1, oc, q_sg[oc % 2])
                    for n, item in enumerate(inject.get(oc, [])):
                        outproj_item(*item, "facc", n % 2 == 1, 1)
                osb = rbp.tile([128, 2 * NT], F32, tag="osb")
                for j in range(2):
                    nc.vector.tensor_copy(osb[:, ts(j, NT)], outp[j])
                norm_chain(h, i0, rp, osb, (0, 1))

            def attention_jsplit(half, h, inject_j1):
                """Last head: process query-tile j=0 fully, then j=1, so
                j=0's normalization chain hides under j=1's pass and the
                it=2 out-projection injects into it; only j=1's chain and
                the it=3 tail stay exposed."""
                i0 = half * 2 * NT
                outp = [
                    psp.tile([128, NT], F32, tag="acc", name=f"outpj{j}")
                    for j in range(2)
                ]
                rp = psp.tile([128, NT], F32, tag="acc", name="rpj")
                osb = rbp.tile([128, 2 * NT], F32, tag="osb")
                for j in range(2):
                    for oc in range(OC):
                        sps = psp.tile(
                            [128, 2 * NT], F32, tag="s", bufs=2, name="spsj"
                        )
                        nc.tensor.matmul(
                            sps[:, :NT],
                            qk_sb[:, 1, h, ds(oc * 128, 128)],
                            qk_sb[:, 0, h, ds(i0 + j * NT, NT)],
                            start=True,
                            stop=True,
                        )
                        p = pp.tile([128, NT], BF16, tag="p")
                        nc.scalar.activation(
                            out=p, in_=sps[:, :NT], func=Exp, bias=0.0,
                            scale=SCALE,
                        )
                        pm = pp.tile([128, NT], BF16, tag="pm")
                        nc.vector.tensor_mul(
                            pm, p, mask_t[(half, oc)][:, ts(j, NT)]
                        )
                        nc.tensor.matmul(
                            outp[j],
                            vp_sb[:, oc, h, :],
                            pm,
                            start=(oc == 0),
                            stop=(oc == OC - 1),
                        )
                        nc.tensor.matmul(
                            rp[32 * j : 32 * j + 1, :],
                            ones_col,
                            pm,
                            start=(oc == 0),
                            stop=(oc == OC - 1),
                        )
                        if j == 1:
                            for n, item in enumerate(inject_j1.get(oc, [])):
                                outproj_item(*item, "facc", n % 2 == 1, 1)
                    nc.vector.tensor_copy(osb[:, ts(j, NT)], outp[j])
                    norm_chain(h, i0, rp, osb, (j,), rowoff=32)

            # half0 attention; its out-projection rides inside half1-h0
            # (items start at oc=4 so half0-h1's normalization chain has
            # finished; the last 4 ocs carry two items each).
            attention(0, 0, {})
            attention(0, 1, {}, prefetch=True)
            half0_items = [(it, dc) for it in (0, 1) for dc in range(DOUT // 128)]
            sched = {oc: [half0_items.pop(0)] for oc in range(4, 12)}
            for oc in range(12, 16):
                sched[oc] = [half0_items.pop(0), half0_items.pop(0)]
            attention(1, 0, sched)
            # last head j-split; it=2 items inject into its j=1 pass
            it2 = [(2, dc) for dc in range(DOUT // 128)]
            attention_jsplit(1, 1, {oc: [it2.pop(0)] for oc in range(8, 16)})
            # tail: it=3 out-projection, alternating DVE/ACT evacuation,
            # 3 PSUM tiles in flight (facc + the two freed s-tag slots).
            for n, dc in enumerate(range(DOUT // 128)):
                if n % 3 == 0:
                    outproj_item(3, dc, "facc", n % 2 == 1, 1)
                else:
                    outproj_item(3, dc, "s", n % 2 == 1, 2)

    return nc


_NC_CACHE = None


def _get_nc():
    global _NC_CACHE
    if _NC_CACHE is None:
        nc = build()
        nc.compile()
        _NC_CACHE = nc
    return _NC_CACHE


def make_in_maps(q, k, v, mask, Wq, bq, Wk, bk, Wv, bv, Wo, bo):
    bf = ml_dtypes.bfloat16
    q = np.asarray(q, np.float32)
    k = np.asarray(k, np.float32)
    v = np.asarray(v, np.float32)
    mask = np.asarray(mask)
    Ws = [np.asarray(w, np.float32) for w in (Wq, Wk, Wv)]
    bs = [np.asarray(b_, np.float32) for b_ in (bq, bk, bv)]
    Wo = np.asarray(Wo, np.float32)
    bo = np.asarray(bo, np.float32)

    xTb, maskTb = [], []
    for b in range(B):
        xTb.append(
            np.ascontiguousarray(np.stack([q[b].T, k[b].T, v[b].T]).astype(bf))
        )
        maskTb.append(
            np.ascontiguousarray(mask[b].T.astype(np.float32)).astype(bf)
        )
    # W[dm, dout] with head h owning columns d*H+h; reshape for tile slicing:
    # Wr[t][kc, p, d, h] = W[kc*128+p, d*H+h]
    Wr = [W.reshape(KC, 128, D, H) for W in Ws]
    br = [b_.reshape(D, H) for b_ in bs]

    ident = np.eye(128, dtype=np.float32).astype(bf)

    in_maps = []
    for c in range(8):
        b = c // 4
        h0 = NH * (c % 4)
        w_core = np.empty((128, 3, KC, NH, D), np.float32)
        for t in range(3):
            for hi in range(NH):
                w_core[:, t, :, hi, :] = Wr[t][:, :, :, h0 + hi].transpose(1, 0, 2)
        # per-partition (d) bias columns for the ACT evacuations
        b_core = np.empty((128, 3, NH), np.float32)
        for t in range(3):
            for hi in range(NH):
                b_core[:, t, hi] = br[t][:, h0 + hi]
        wo_core = np.stack([Wo[h0 + hi :: H, :] for hi in range(NH)], axis=1)
        bo_core = bo if c % 4 == 0 else np.zeros_like(bo)
        in_maps.append(
            {
                "xT": xTb[b],
                "ident": ident,
                "maskT": maskTb[b],
                "w_qkv": np.ascontiguousarray(w_core).astype(bf),
                "b_qkv": np.ascontiguousarray(b_core),
                "wo": np.ascontiguousarray(wo_core).astype(bf),
                "bo": np.ascontiguousarray(bo_core.reshape(DOUT // 128, 128).T),
            }
        )
    return in_maps


def unshard(results):
    out = np.zeros((B, DOUT, S), np.float32)
    for c in range(8):
        out[c // 4] += np.asarray(results[c]["outT"], np.float32)
    return np.ascontiguousarray(out.transpose(0, 2, 1))


def kernel(**inputs):
    in_maps = make_in_maps(**inputs)
    nc = _get_nc()
    res = run_bass_kernel_spmd(nc, in_maps, core_ids=list(range(8)))
    return unshard(res.results)


# revision 21
# speedup vs baseline: 1.0378x; 1.0378x over previous
"""Trainium2 Bass kernel for nn_MultiHeadAttention (B=2, S=2048, DM=1024, H=8).

Sharding: data-parallel on batch x tensor-parallel on heads.
Core c in 0..7 handles batch b = c//4 and heads {2*(c%4), 2*(c%4)+1}.
Each core computes its two heads' full attention and the partial
out-projection (a 1024x2048 partial sum in fp16); the host adds the 4
partials per batch and transposes back to (S, DOUT).

v3 structure (why):
  - the kernel is PE-streaming-bound (~140us of bf16 matmul columns);
    everything else is scheduled to keep the PE queue from ever waiting.
  - x ring of 14 bufs; w/x interleaved in need order on 3 DMA queues.
  - mask is bf16 (DVE mask-multiply stays in 2x mode; fp8 operands
    halve DVE throughput) but shipped as 32 half-column tiles
    [128,1024] through a 20-buf ring: only half0's 4.2MB must land
    before attention starts; half1's tiles stream during half0's
    attention when HBM is otherwise idle.
  - scalar queue carries only ungated DMAs (its ACT evacuations can
    never sit behind a ring-gated DMA semaphore); gated x-t2 and
    half1-mask DMAs live on sync/gpsimd.
  - PSUM bias-evacuation on ACT (Identity + per-partition bias AP).
  - reciprocal on a [128,8]-repacked layout (DVE reciprocal cost is
    its free-dim size: [1,512] would be ~4us, [128,8] is ~0.3us).
  - out-projection of half0 is software-pipelined one tile per oc
    into half1-h0's attention loop so its PE/DVE work hides under the
    attention stream instead of serializing after it; half1's
    out-projection forms the tail with alternating DVE/ACT evacs.
  - outT in fp16 (halves output DMA; host sums partials in fp32).
PSUM budget (8 banks): tag acc 3 + tag s 2x[128,1024] = 4 + facc 1.
Projections therefore accumulate 2 query-tiles at a time (2 groups).
"""

import sys

sys.path.insert(0, "/opt/trn_rl_repo")

import numpy as np
import ml_dtypes

import concourse.bass as bass
import concourse.tile as tile
from concourse import bacc, mybir
from concourse.bass import ts, ds
from concourse.bass_utils import run_bass_kernel_spmd

BF16 = mybir.dt.bfloat16
F32 = mybir.dt.float32
F16 = mybir.dt.float16
Exp = mybir.ActivationFunctionType.Exp
Ident = mybir.ActivationFunctionType.Identity

B, S, DM, H, DOUT = 2, 2048, 1024, 8, 1024
D = DM // H            # 128 head dim
NH = 2                 # heads per core
KC = DM // 128         # 8 contraction chunks for projections
OC = S // 128          # 16 key chunks
NT = 512               # PSUM-bank-sized free tile (fp32)
IT = S // NT           # 4 query tiles
SCALE = float(1.0 / np.sqrt(np.float32(D)))


def build():
    nc = bacc.Bacc(None, target_bir_lowering=False)

    xT = nc.dram_tensor("xT", [3, DM, S], BF16, kind="ExternalInput")
    maskT = nc.dram_tensor("maskT", [S, S], BF16, kind="ExternalInput")
    w_qkv = nc.dram_tensor("w_qkv", [128, 3, KC, NH, D], BF16, kind="ExternalInput")
    b_qkv = nc.dram_tensor("b_qkv", [128, 3, NH], F32, kind="ExternalInput")
    wo = nc.dram_tensor("wo", [D, NH, DOUT], BF16, kind="ExternalInput")
    ident = nc.dram_tensor("ident", [128, 128], BF16, kind="ExternalInput")
    bo = nc.dram_tensor("bo", [128, DOUT // 128], F32, kind="ExternalInput")
    outT = nc.dram_tensor("outT", [DOUT, S], F16, kind="ExternalOutput")

    q_sg = [nc.sync, nc.gpsimd]
    q_sgs = [nc.sync, nc.gpsimd, nc.scalar]

    with tile.TileContext(nc) as tc:
        with (
            tc.tile_pool(name="const", bufs=1) as constp,
            tc.tile_pool(name="xin", bufs=16) as xp,
            tc.tile_pool(name="mask", bufs=20) as mp,
            tc.tile_pool(name="ptile", bufs=4) as pp,
            tc.tile_pool(name="rb", bufs=2) as rbp,
            tc.tile_pool(name="fout", bufs=4) as fop,
            tc.tile_pool(name="psum", bufs=3, space="PSUM") as psp,
            tc.tile_pool(name="dram", bufs=2, space="DRAM") as dramp,
        ):
            # ---- small constants (scalar queue; ungated, tiny) ----
            b_sb = constp.tile([128, 3, NH], F32)
            nc.scalar.dma_start(out=b_sb, in_=b_qkv[:])
            bo_sb = constp.tile([128, DOUT // 128], F32)
            nc.scalar.dma_start(out=bo_sb, in_=bo[:])
            ident_sb = constp.tile([128, 128], BF16)
            nc.scalar.dma_start(out=ident_sb, in_=ident[:])
            ones_col = constp.tile([128, 1], BF16)
            nc.vector.memset(ones_col, 1.0)

            # ---- weights + x tiles interleaved in need order ----
            w_sb = constp.tile([128, 3, KC, NH, D], BF16)
            wo_sb = constp.tile([D, NH, DOUT], BF16)
            xts_all = []
            for t in range(3):
                row = []
                for k in range(KC):
                    qi = t * KC + k
                    if t < 2:
                        q_w = q_sgs[(qi + 1) % 3]
                        q_x = q_sgs[qi % 3]
                    else:
                        # ring-gated x-t2: sync/gpsimd only; its (ungated)
                        # weights go early on scalar.
                        q_w = nc.scalar
                        q_x = q_sg[qi % 2]
                    q_w.dma_start(out=w_sb[:, t, k, :, :], in_=w_qkv[:, t, k, :, :])
                    xt = xp.tile([128, S], BF16, tag="x", name=f"x{t}_{k}")
                    q_x.dma_start(out=xt, in_=xT[t, k * 128 : (k + 1) * 128, :])
                    row.append(xt)
                xts_all.append(row)

            # mask half-column tiles [128,1024] keyed (half, oc). half0's 16
            # are queued here behind x-t2 (sync/gpsimd, few on scalar);
            # half1's are allocated+issued lazily inside half0-h1's
            # attention loop so they enter the in-order queues after the
            # half0 reciprocal-chain DMAs and stream while HBM is idle.
            mask_t = {}

            def mask_fetch(half, oc, qm):
                mt = mp.tile([128, 2 * NT], BF16, tag="m", name=f"m{half}_{oc}")
                qm.dma_start(
                    out=mt,
                    in_=maskT[
                        oc * 128 : (oc + 1) * 128,
                        half * 2 * NT : (half + 1) * 2 * NT,
                    ],
                )
                mask_t[(half, oc)] = mt

            for oc in range(OC):
                mask_fetch(0, oc, q_sg[oc % 2])
            nc.scalar.dma_start(out=wo_sb, in_=wo[:])

            # ---- Q/K projections: qk_sb[d, t, h, s] (2 query-tile groups,
            #      3-buf PSUM acc ring); bias added on ACT evac ----
            qk_sb = constp.tile([128, 2, NH, S], BF16)
            for t in range(2):
                xts = xts_all[t]
                for h in range(NH):
                    for g in range(2):
                        acc = [
                            psp.tile([128, NT], F32, tag="acc", name=f"acc{g}{i}")
                            for i in range(2)
                        ]
                        for k in range(KC):
                            for i in range(2):
                                nc.tensor.matmul(
                                    acc[i],
                                    w_sb[:, t, k, h, :],
                                    xts[k][:, ts(2 * g + i, NT)],
                                    start=(k == 0),
                                    stop=(k == KC - 1),
                                )
                        for i in range(2):
                            nc.scalar.activation(
                                out=qk_sb[:, t, h, ts(2 * g + i, NT)],
                                in_=acc[i],
                                func=Ident,
                                bias=b_sb[:, t, h : h + 1],
                            )

            # ---- V projection via VpT + PE transpose: vp_sb[s%128, oc, h, d] ----
            vpt_sb = constp.tile([128, NH, S], BF16)  # [d, h, s] transient
            vp_sb = constp.tile([128, OC, NH, D], BF16)
            xts = xts_all[2]
            for h in range(NH):
                for g in range(2):
                    acc = [
                        psp.tile([128, NT], F32, tag="acc", name=f"vacc{g}{i}")
                        for i in range(2)
                    ]
                    for k in range(KC):
                        for i in range(2):
                            nc.tensor.matmul(
                                acc[i],
                                w_sb[:, 2, k, h, :],
                                xts[k][:, ts(2 * g + i, NT)],
                                start=(k == 0),
                                stop=(k == KC - 1),
                            )
                    for i in range(2):
                        nc.scalar.activation(
                            out=vpt_sb[:, h, ts(2 * g + i, NT)],
                            in_=acc[i],
                            func=Ident,
                            bias=b_sb[:, 2, h : h + 1],
                        )
                for oc in range(OC):
                    tps = psp.tile([128, D], BF16, tag="s", bufs=2, name="tps")
                    nc.tensor.transpose(
                        tps, vpt_sb[:, h, ds(oc * 128, 128)], ident_sb
                    )
                    nc.vector.tensor_copy(vp_sb[:, oc, h, :], tps)

            outn_sb = constp.tile([128, NH, S], BF16)

            def outproj_item(it, dc, facc_tag, on_act, facc_bufs=1):
                """One out-projection tile: facc = sum_h wo_h^T outn_h."""
                facc = psp.tile(
                    [128, NT], F32, tag=facc_tag, bufs=facc_bufs, name="facc"
                )
                for h in range(NH):
                    nc.tensor.matmul(
                        facc,
                        wo_sb[:, h, ds(dc * 128, 128)],
                        outn_sb[:, h, ts(it, NT)],
                        start=(h == 0),
                        stop=(h == NH - 1),
                    )
                fsb = fop.tile([128, NT], F16, tag="f")
                if on_act:
                    nc.scalar.activation(
                        out=fsb, in_=facc, func=Ident,
                        bias=bo_sb[:, dc : dc + 1],
                    )
                else:
                    nc.vector.tensor_scalar_add(
                        out=fsb, in0=facc, scalar1=bo_sb[:, dc : dc + 1]
                    )
                q_sg[dc % 2].dma_start(
                    out=outT[dc * 128 : (dc + 1) * 128, ts(it, NT)], in_=fsb
                )

            def norm_chain(h, i0, rp, osb, jset, rowoff=32):
                """Rowsum rows (PSUM) -> DRAM -> [128,w] repack -> reciprocal
                -> DRAM -> per-j broadcast -> outn multiply. DVE reciprocal
                cost is its free-dim size, hence the repack."""
                w = 4 * len(jset)
                r2 = rbp.tile([33, NT], F32, tag="r2")
                for j in jset:
                    nc.vector.tensor_copy(
                        r2[rowoff * j : rowoff * j + 1, :],
                        rp[rowoff * j : rowoff * j + 1, :],
                    )
                rd = dramp.tile([len(jset), NT], F32, tag="rd")
                for n, j in enumerate(jset):
                    q_sg[j].dma_start(
                        out=rd[n : n + 1, :],
                        in_=r2[rowoff * j : rowoff * j + 1, :],
                    )
                rseg = rbp.tile([128, 8], F32, tag="rseg")
                nc.sync.dma_start(
                    out=rseg[:, :w],
                    in_=rd[:].rearrange("a b -> (a b)").rearrange(
                        "(p j) -> p j", p=128
                    ),
                )
                nc.vector.reciprocal(rseg[:, :w], rseg[:, :w])
                rd2 = dramp.tile([len(jset), NT], F32, tag="rd2")
                nc.gpsimd.dma_start(
                    out=rd2[:].rearrange("a b -> (a b)").rearrange(
                        "(p j) -> p j", p=128
                    ),
                    in_=rseg[:, :w],
                )
                for n, j in enumerate(jset):
                    rbc = rbp.tile([128, NT], F32, tag="rbc", bufs=2)
                    q_sg[j].dma_start(
                        out=rbc, in_=rd2[n : n + 1, :].to_broadcast([128, NT])
                    )
                    nc.vector.tensor_mul(
                        outn_sb[:, h, ds(i0 + j * NT, NT)],
                        osb[:, ts(j, NT)],
                        rbc,
                    )

            LAG = 2  # oc-steps between scores emission and PV/rowsum use;
            # covers the scores->exp->mask chain so the PE never stalls.

            def attention(half, h, inject, prefetch=False):
                """One head's attention over 1024 queries, software-pipelined:
                at step oc the PE runs scores(oc) and PV/rowsum(oc-LAG), whose
                exp+mask completed ~2 steps ago. `inject` maps oc -> deferred
                out-projection items; `prefetch` issues the next half's mask
                DMAs one per oc."""
                i0 = half * 2 * NT
                outp = [
                    psp.tile([128, NT], F32, tag="acc", name=f"outp{j}")
                    for j in range(2)
                ]
                rp = psp.tile([128, NT], F32, tag="acc", name="rp")
                pms = {}
                for step in range(OC + LAG):
                    if step < OC:
                        oc = step
                        sps = psp.tile(
                            [128, 2 * NT], F32, tag="s", bufs=2, name="sps"
                        )
                        for j in range(2):
                            nc.tensor.matmul(
                                sps[:, ts(j, NT)],
                                qk_sb[:, 1, h, ds(oc * 128, 128)],
                                qk_sb[:, 0, h, ds(i0 + j * NT, NT)],
                                start=True,
                                stop=True,
                            )
                        p = pp.tile([128, 2 * NT], BF16, tag="p")
                        nc.scalar.activation(
                            out=p, in_=sps, func=Exp, bias=0.0, scale=SCALE
                        )
                        pm = pp.tile([128, 2 * NT], BF16, tag="pm")
                        nc.vector.tensor_mul(pm, p, mask_t[(half, oc)])
                        pms[oc] = pm
                        if prefetch:
                            mask_fetch(half + 1, oc, q_sg[oc % 2])
                    if step >= LAG:
                        oc = step - LAG
                        pm = pms.pop(oc)
                        for j in range(2):
                            nc.tensor.matmul(
                                outp[j],
                                vp_sb[:, oc, h, :],
                                pm[:, ts(j, NT)],
                                start=(oc == 0),
                                stop=(oc == OC - 1),
                            )
                        for j in range(2):
                            nc.tensor.matmul(
                                rp[32 * j : 32 * j + 1, :],
                                ones_col,
                                pm[:, ts(j, NT)],
                                start=(oc == 0),
                                stop=(oc == OC - 1),
                            )
                        for n, item in enumerate(inject.get(oc, [])):
                            outproj_item(*item, "facc", n % 2 == 1, 1)
                osb = rbp.tile([128, 2 * NT], F32, tag="osb")
                for j in range(2):
                    nc.vector.tensor_copy(osb[:, ts(j, NT)], outp[j])
                norm_chain(h, i0, rp, osb, (0, 1))

            def attention_jsplit(half, h, inject_j1):
                """Last head: process query-tile j=0 fully, then j=1 (both
                software-pipelined), so j=0's normalization chain hides under
                j=1's pass and the it=2 out-projection injects into it; only
                j=1's chain and the it=3 tail stay exposed."""
                i0 = half * 2 * NT
                outp = [
                    psp.tile([128, NT], F32, tag="acc", name=f"outpj{j}")
                    for j in range(2)
                ]
                rp = psp.tile([128, NT], F32, tag="acc", name="rpj")
                osb = rbp.tile([128, 2 * NT], F32, tag="osb")
                for j in range(2):
                    pms = {}
                    for step in range(OC + LAG):
                        if step < OC:
                            oc = step
                            sps = psp.tile(
                                [128, 2 * NT], F32, tag="s", bufs=2, name="spsj"
                            )
                            nc.tensor.matmul(
                                sps[:, :NT],
                                qk_sb[:, 1, h, ds(oc * 128, 128)],
                                qk_sb[:, 0, h, ds(i0 + j * NT, NT)],
                                start=True,
                                stop=True,
                            )
                            p = pp.tile([128, NT], BF16, tag="p")
                            nc.scalar.activation(
                                out=p, in_=sps[:, :NT], func=Exp, bias=0.0,
                                scale=SCALE,
                            )
                            pm = pp.tile([128, NT], BF16, tag="pm")
                            nc.vector.tensor_mul(
                                pm, p, mask_t[(half, oc)][:, ts(j, NT)]
                            )
                            pms[oc] = pm
                        if step >= LAG:
                            oc = step - LAG
                            pm = pms.pop(oc)
                            nc.tensor.matmul(
                                outp[j],
                                vp_sb[:, oc, h, :],
                                pm,
                                start=(oc == 0),
                                stop=(oc == OC - 1),
                            )
                            nc.tensor.matmul(
                                rp[32 * j : 32 * j + 1, :],
                                ones_col,
                                pm,
                                start=(oc == 0),
                                stop=(oc == OC - 1),
                            )
                            if j == 1:
                                for n, item in enumerate(inject_j1.get(oc, [])):
                                    outproj_item(*item, "facc", n % 2 == 1, 1)
                    nc.vector.tensor_copy(osb[:, ts(j, NT)], outp[j])
                    norm_chain(h, i0, rp, osb, (j,), rowoff=32)

            # half0 attention; its out-projection rides inside half1-h0
            # (items start at oc=4 so half0-h1's normalization chain has
            # finished; the last 4 ocs carry two items each).
            attention(0, 0, {})
            attention(0, 1, {}, prefetch=True)
            half0_items = [(it, dc) for it in (0, 1) for dc in range(DOUT // 128)]
            sched = {oc: [half0_items.pop(0)] for oc in range(4, 12)}
            for oc in range(12, 16):
                sched[oc] = [half0_items.pop(0), half0_items.pop(0)]
            attention(1, 0, sched)
            # last head j-split; it=2 items inject into its j=1 pass
            it2 = [(2, dc) for dc in range(DOUT // 128)]
            attention_jsplit(1, 1, {oc: [it2.pop(0)] for oc in range(8, 16)})
            # tail: it=3 out-projection, alternating DVE/ACT evacuation,
            # 3 PSUM tiles in flight (facc + the two freed s-tag slots).
            for n, dc in enumerate(range(DOUT // 128)):
                if n % 3 == 0:
                    outproj_item(3, dc, "facc", n % 2 == 1, 1)
                else:
                    outproj_item(3, dc, "s", n % 2 == 1, 2)

    return nc


_NC_CACHE = None


def _get_nc():
    global _NC_CACHE
    if _NC_CACHE is None:
        nc = build()
        nc.compile()
        _NC_CACHE = nc
    return _NC_CACHE


def make_in_maps(q, k, v, mask, Wq, bq, Wk, bk, Wv, bv, Wo, bo):
    bf = ml_dtypes.bfloat16
    q = np.asarray(q, np.float32)
    k = np.asarray(k, np.float32)
    v = np.asarray(v, np.float32)
    mask = np.asarray(mask)
    Ws = [np.asarray(w, np.float32) for w in (Wq, Wk, Wv)]
    bs = [np.asarray(b_, np.float32) for b_ in (bq, bk, bv)]
    Wo = np.asarray(Wo, np.float32)
    bo = np.asarray(bo, np.float32)

    xTb, maskTb = [], []
    for b in range(B):
        xTb.append(
            np.ascontiguousarray(np.stack([q[b].T, k[b].T, v[b].T]).astype(bf))
        )
        maskTb.append(
            np.ascontiguousarray(mask[b].T.astype(np.float32)).astype(bf)
        )
    # W[dm, dout] with head h owning columns d*H+h; reshape for tile slicing:
    # Wr[t][kc, p, d, h] = W[kc*128+p, d*H+h]
    Wr = [W.reshape(KC, 128, D, H) for W in Ws]
    br = [b_.reshape(D, H) for b_ in bs]

    ident = np.eye(128, dtype=np.float32).astype(bf)

    in_maps = []
    for c in range(8):
        b = c // 4
        h0 = NH * (c % 4)
        w_core = np.empty((128, 3, KC, NH, D), np.float32)
        for t in range(3):
            for hi in range(NH):
                w_core[:, t, :, hi, :] = Wr[t][:, :, :, h0 + hi].transpose(1, 0, 2)
        # per-partition (d) bias columns for the ACT evacuations
        b_core = np.empty((128, 3, NH), np.float32)
        for t in range(3):
            for hi in range(NH):
                b_core[:, t, hi] = br[t][:, h0 + hi]
        wo_core = np.stack([Wo[h0 + hi :: H, :] for hi in range(NH)], axis=1)
        bo_core = bo if c % 4 == 0 else np.zeros_like(bo)
        in_maps.append(
            {
                "xT": xTb[b],
                "ident": ident,
                "maskT": maskTb[b],
                "w_qkv": np.ascontiguousarray(w_core).astype(bf),
                "b_qkv": np.ascontiguousarray(b_core),
                "wo": np.ascontiguousarray(wo_core).astype(bf),
                "bo": np.ascontiguousarray(bo_core.reshape(DOUT // 128, 128).T),
            }
        )
    return in_maps


def unshard(results):
    out = np.zeros((B, DOUT, S), np.float32)
    for c in range(8):
        out[c // 4] += np.asarray(results[c]["outT"], np.float32)
    return np.ascontiguousarray(out.transpose(0, 2, 1))


def kernel(**inputs):
    in_maps = make_in_maps(**inputs)
    nc = _get_nc()
    res = run_bass_kernel_spmd(nc, in_maps, core_ids=list(range(8)))
    return unshard(res.results)


# revision 23
# speedup vs baseline: 1.1274x; 1.0863x over previous
"""Trainium2 Bass kernel for nn_MultiHeadAttention (B=2, S=2048, DM=1024, H=8).

Sharding: data-parallel on batch x tensor-parallel on heads.
Core c in 0..7 handles batch b = c//4 and heads {2*(c%4), 2*(c%4)+1}.
Each core computes its two heads' full attention and the partial
out-projection (a 1024x2048 partial sum in fp16); the host adds the 4
partials per batch and transposes back to (S, DOUT).

v6 structure (why):
  - DMA packet efficiency is the input-phase bottleneck: each DMA
    engine moves one contiguous run per ~(100ns + bytes/rate), so 4KB
    runs yield only ~20GB/s/engine (~310GB/s total). All large inputs
    are therefore shipped with >=8KB contiguous per partition:
      * x as [128, 2, S] pair-tiles (DRAM rows 2p,2p+1 on partition p;
        the row permutation is absorbed into the projection weights).
      * all projection weights in ONE DMA, 12KB/partition contiguous.
      * mask as [128, 2, 1024] pair-tiles; the key permutation is
        realized on-chip with stride-2 stationary/transpose views, so
        attention key-chunk ordinal oc = 2*oc2 + i covers keys
        {256*oc2 + 2p + i}.
  - mask tiles stream through a ring: half0's before attention,
    half1's prefetched during half0-h1's loop when HBM is idle.
  - attention is software-pipelined with a 2-chunk lag (scores(oc)
    and PV/rowsum(oc-2) per step) so the scores->exp->mask chain
    never stalls the PE and its clock stays at max p-state.
  - PSUM bias-evacuation on ACT (Identity + per-partition bias AP);
    reciprocal on a [128,8] repack; out-projection of half0 pipelined
    into half1-h0; last head split by query-tile j so its rowsum
    chains overlap and it=2 injects into the j=1 pass; it=3 tail
    alternates DVE/ACT evacuation; outT in fp16.
PSUM budget (8 banks): tag acc 3 + tag s 2x[128,1024] = 4 + facc 1.
"""

import sys

sys.path.insert(0, "/opt/trn_rl_repo")

import numpy as np
import ml_dtypes

import concourse.bass as bass
import concourse.tile as tile
from concourse import bacc, mybir
from concourse.bass import ts, ds
from concourse.bass_utils import run_bass_kernel_spmd

BF16 = mybir.dt.bfloat16
F32 = mybir.dt.float32
F16 = mybir.dt.float16
Exp = mybir.ActivationFunctionType.Exp
Ident = mybir.ActivationFunctionType.Identity

B, S, DM, H, DOUT = 2, 2048, 1024, 8, 1024
D = DM // H            # 128 head dim
NH = 2                 # heads per core
KC = DM // 128         # 8 contraction chunks for projections
KP = KC // 2           # 4 paired x tiles per tensor
OC = S // 128          # 16 key chunks
OC2 = OC // 2          # 8 paired mask tiles per half
NT = 512               # PSUM-bank-sized free tile (fp32)
IT = S // NT           # 4 query tiles
SCALE = float(1.0 / np.sqrt(np.float32(D)))


def build():
    nc = bacc.Bacc(None, target_bir_lowering=False)

    xT = nc.dram_tensor("xT", [3, KP, 128, 2, S], BF16, kind="ExternalInput")
    maskT = nc.dram_tensor("maskT", [2, OC2, 128, 2, 2 * NT], BF16,
                           kind="ExternalInput")
    w_qkv = nc.dram_tensor("w_qkv", [128, 3, KP, 2, NH, D], BF16,
                           kind="ExternalInput")
    b_qkv = nc.dram_tensor("b_qkv", [128, 3, NH], F32, kind="ExternalInput")
    wo = nc.dram_tensor("wo", [D, NH, DOUT], BF16, kind="ExternalInput")
    ident = nc.dram_tensor("ident", [128, 128], BF16, kind="ExternalInput")
    bo = nc.dram_tensor("bo", [128, DOUT // 128], F32, kind="ExternalInput")
    outT = nc.dram_tensor("outT", [DOUT, S], F16, kind="ExternalOutput")

    q_sg = [nc.sync, nc.gpsimd]
    q_sgs = [nc.sync, nc.gpsimd, nc.scalar]

    with tile.TileContext(nc) as tc:
        with (
            tc.tile_pool(name="const", bufs=1) as constp,
            tc.tile_pool(name="xin", bufs=8) as xp,
            tc.tile_pool(name="mask", bufs=10) as mp,
            tc.tile_pool(name="ptile", bufs=4) as pp,
            tc.tile_pool(name="rb", bufs=2) as rbp,
            tc.tile_pool(name="fout", bufs=4) as fop,
            tc.tile_pool(name="psum", bufs=3, space="PSUM") as psp,
            tc.tile_pool(name="dram", bufs=2, space="DRAM") as dramp,
        ):
            # ---- small constants + the single big weight DMA ----
            b_sb = constp.tile([128, 3, NH], F32)
            nc.scalar.dma_start(out=b_sb, in_=b_qkv[:])
            bo_sb = constp.tile([128, DOUT // 128], F32)
            nc.scalar.dma_start(out=bo_sb, in_=bo[:])
            ident_sb = constp.tile([128, 128], BF16)
            nc.scalar.dma_start(out=ident_sb, in_=ident[:])
            ones_col = constp.tile([128, 1], BF16)
            nc.vector.memset(ones_col, 1.0)
            w_sb = constp.tile([128, 3, KP, 2, NH, D], BF16)
            nc.scalar.dma_start(out=w_sb, in_=w_qkv[:])

            # ---- x pair-tiles in need order ----
            wo_sb = constp.tile([D, NH, DOUT], BF16)
            xts_all = []
            for t in range(3):
                row = []
                for j2 in range(KP):
                    qi = t * KP + j2
                    q_x = q_sgs[qi % 3] if t < 2 else q_sg[qi % 2]
                    xt = xp.tile([128, 2, S], BF16, tag="x", name=f"x{t}_{j2}")
                    q_x.dma_start(out=xt, in_=xT[t, j2])
                    row.append(xt)
                xts_all.append(row)

            # mask pair-tiles [128, 2, 1024] keyed (half, oc2): half0's are
            # queued behind x-t2; half1's stream during half0-h1's loop.
            mask_t = {}

            def mask_fetch(half, oc2, qm):
                mt = mp.tile([128, 2, 2 * NT], BF16, tag="m",
                             name=f"m{half}_{oc2}")
                qm.dma_start(out=mt, in_=maskT[half, oc2])
                mask_t[(half, oc2)] = mt

            for oc2 in range(OC2):
                mask_fetch(0, oc2, q_sg[oc2 % 2])
            nc.scalar.dma_start(out=wo_sb, in_=wo[:])

            # ---- Q/K projections: qk_sb[d, t, h, s] (2 query-tile groups,
            #      3-buf PSUM acc ring); bias added on ACT evac ----
            qk_sb = constp.tile([128, 2, NH, S], BF16)
            for t in range(2):
                xts = xts_all[t]
                for h in range(NH):
                    for g in range(2):
                        acc = [
                            psp.tile([128, NT], F32, tag="acc", name=f"ac{g}{i}")
                            for i in range(2)
                        ]
                        for j2 in range(KP):
                            for i in range(2):
                                for i2 in range(2):
                                    nc.tensor.matmul(
                                        acc[i2],
                                        w_sb[:, t, j2, i, h, :],
                                        xts[j2][:, i, ts(2 * g + i2, NT)],
                                        start=(j2 == 0 and i == 0),
                                        stop=(j2 == KP - 1 and i == 1),
                                    )
                        for i2 in range(2):
                            nc.scalar.activation(
                                out=qk_sb[:, t, h, ts(2 * g + i2, NT)],
                                in_=acc[i2],
                                func=Ident,
                                bias=b_sb[:, t, h : h + 1],
                            )

            # stride-2 key-chunk view: kview(h)[oc2, p, i] = K[d, key
            # 256*oc2 + 2p + i]; chunk ordinal oc = 2*oc2 + i.
            def kchunk(h, oc):
                v = qk_sb[:, 1, h, :].rearrange(
                    "d (a k i) -> d a k i", a=OC2, i=2
                )
                return v[:, oc // 2, :, oc % 2]

            # ---- V projection via VpT + PE transpose: vp_sb[key%128, oc, h, d]
            #      with the same permuted key-chunk ordinals ----
            vpt_sb = constp.tile([128, NH, S], BF16)  # [d, h, s] transient
            vp_sb = constp.tile([128, OC, NH, D], BF16)
            xts = xts_all[2]
            for h in range(NH):
                for g in range(2):
                    acc = [
                        psp.tile([128, NT], F32, tag="acc", name=f"va{g}{i}")
                        for i in range(2)
                    ]
                    for j2 in range(KP):
                        for i in range(2):
                            for i2 in range(2):
                                nc.tensor.matmul(
                                    acc[i2],
                                    w_sb[:, 2, j2, i, h, :],
                                    xts[j2][:, i, ts(2 * g + i2, NT)],
                                    start=(j2 == 0 and i == 0),
                                    stop=(j2 == KP - 1 and i == 1),
                                )
                    for i2 in range(2):
                        nc.scalar.activation(
                            out=vpt_sb[:, h, ts(2 * g + i2, NT)],
                            in_=acc[i2],
                            func=Ident,
                            bias=b_sb[:, 2, h : h + 1],
                        )
                vview = vpt_sb[:, h, :].rearrange(
                    "d (a k i) -> d a k i", a=OC2, i=2
                )
                for oc in range(OC):
                    tps = psp.tile([128, D], BF16, tag="s", bufs=2, name="tps")
                    nc.tensor.transpose(
                        tps, vview[:, oc // 2, :, oc % 2], ident_sb
                    )
                    nc.vector.tensor_copy(vp_sb[:, oc, h, :], tps)

            outn_sb = constp.tile([128, NH, S], BF16)

            def outproj_item(it, dc, facc_tag, on_act, facc_bufs=1):
                """One out-projection tile: facc = sum_h wo_h^T outn_h."""
                facc = psp.tile(
                    [128, NT], F32, tag=facc_tag, bufs=facc_bufs, name="facc"
                )
                for h in range(NH):
                    nc.tensor.matmul(
                        facc,
                        wo_sb[:, h, ds(dc * 128, 128)],
                        outn_sb[:, h, ts(it, NT)],
                        start=(h == 0),
                        stop=(h == NH - 1),
                    )
                fsb = fop.tile([128, NT], F16, tag="f")
                if on_act:
                    nc.scalar.activation(
                        out=fsb, in_=facc, func=Ident,
                        bias=bo_sb[:, dc : dc + 1],
                    )
                else:
                    nc.vector.tensor_scalar_add(
                        out=fsb, in0=facc, scalar1=bo_sb[:, dc : dc + 1]
                    )
                q_sg[dc % 2].dma_start(
                    out=outT[dc * 128 : (dc + 1) * 128, ts(it, NT)], in_=fsb
                )

            def norm_chain(h, i0, rp, osb, jset, rowoff=32):
                """Rowsum rows (PSUM) -> SBUF -> DRAM -> [128,w] repack ->
                reciprocal -> DRAM -> per-j broadcast -> outn multiply."""
                w = 4 * len(jset)
                r2 = rbp.tile([33, NT], F32, tag="r2")
                for j in jset:
                    nc.vector.tensor_copy(
                        r2[rowoff * j : rowoff * j + 1, :],
                        rp[rowoff * j : rowoff * j + 1, :],
                    )
                rd = dramp.tile([len(jset), NT], F32, tag="rd")
                for n, j in enumerate(jset):
                    q_sg[j].dma_start(
                        out=rd[n : n + 1, :],
                        in_=r2[rowoff * j : rowoff * j + 1, :],
                    )
                rseg = rbp.tile([128, 8], F32, tag="rseg")
                nc.sync.dma_start(
                    out=rseg[:, :w],
                    in_=rd[:].rearrange("a b -> (a b)").rearrange(
                        "(p j) -> p j", p=128
                    ),
                )
                nc.vector.reciprocal(rseg[:, :w], rseg[:, :w])
                rd2 = dramp.tile([len(jset), NT], F32, tag="rd2")
                nc.gpsimd.dma_start(
                    out=rd2[:].rearrange("a b -> (a b)").rearrange(
                        "(p j) -> p j", p=128
                    ),
                    in_=rseg[:, :w],
                )
                for n, j in enumerate(jset):
                    rbc = rbp.tile([128, NT], F32, tag="rbc", bufs=2)
                    q_sg[j].dma_start(
                        out=rbc, in_=rd2[n : n + 1, :].to_broadcast([128, NT])
                    )
                    nc.vector.tensor_mul(
                        outn_sb[:, h, ds(i0 + j * NT, NT)],
                        osb[:, ts(j, NT)],
                        rbc,
                    )

            LAG = 2  # oc-steps between scores emission and PV/rowsum use

            def attention(half, h, inject, prefetch=False):
                """One head's attention over 1024 queries, software-pipelined:
                at step oc the PE runs scores(oc) and PV/rowsum(oc-LAG)."""
                i0 = half * 2 * NT
                outp = [
                    psp.tile([128, NT], F32, tag="acc", name=f"outp{j}")
                    for j in range(2)
                ]
                rp = psp.tile([128, NT], F32, tag="acc", name="rp")
                pms = {}
                for step in range(OC + LAG):
                    if step < OC:
                        oc = step
                        sps = psp.tile(
                            [128, 2 * NT], F32, tag="s", bufs=2, name="sps"
                        )
                        for j in range(2):
                            nc.tensor.matmul(
                                sps[:, ts(j, NT)],
                                kchunk(h, oc),
                                qk_sb[:, 0, h, ds(i0 + j * NT, NT)],
                                start=True,
                                stop=True,
                            )
                        p = pp.tile([128, 2 * NT], BF16, tag="p")
                        nc.scalar.activation(
                            out=p, in_=sps, func=Exp, bias=0.0, scale=SCALE
                        )
                        pm = pp.tile([128, 2 * NT], BF16, tag="pm")
                        nc.vector.tensor_mul(
                            pm, p, mask_t[(half, oc // 2)][:, oc % 2, :]
                        )
                        pms[oc] = pm
                        if prefetch and oc % 2 == 0:
                            mask_fetch(half + 1, oc // 2, q_sg[(oc // 2) % 2])
                    if step >= LAG:
                        oc = step - LAG
                        pm = pms.pop(oc)
                        for j in range(2):
                            nc.tensor.matmul(
                                outp[j],
                                vp_sb[:, oc, h, :],
                                pm[:, ts(j, NT)],
                                start=(oc == 0),
                                stop=(oc == OC - 1),
                            )
                        for j in range(2):
                            nc.tensor.matmul(
                                rp[32 * j : 32 * j + 1, :],
                                ones_col,
                                pm[:, ts(j, NT)],
                                start=(oc == 0),
                                stop=(oc == OC - 1),
                            )
                        for n, item in enumerate(inject.get(oc, [])):
                            outproj_item(*item, "facc", n % 2 == 1, 1)
                osb = rbp.tile([128, 2 * NT], F32, tag="osb")
                for j in range(2):
                    nc.vector.tensor_copy(osb[:, ts(j, NT)], outp[j])
                norm_chain(h, i0, rp, osb, (0, 1))

            def attention_jsplit(half, h, inject_j1):
                """Last head: query-tile j=0 fully, then j=1 (both pipelined),
                so j=0's chain hides under j=1 and it=2 injects there."""
                i0 = half * 2 * NT
                outp = [
                    psp.tile([128, NT], F32, tag="acc", name=f"outpj{j}")
                    for j in range(2)
                ]
                rp = psp.tile([128, NT], F32, tag="acc", name="rpj")
                osb = rbp.tile([128, 2 * NT], F32, tag="osb")
                for j in range(2):
                    pms = {}
                    for step in range(OC + LAG):
                        if step < OC:
                            oc = step
                            sps = psp.tile(
                                [128, 2 * NT], F32, tag="s", bufs=2, name="spsj"
                            )
                            nc.tensor.matmul(
                                sps[:, :NT],
                                kchunk(h, oc),
                                qk_sb[:, 0, h, ds(i0 + j * NT, NT)],
                                start=True,
                                stop=True,
                            )
                            p = pp.tile([128, NT], BF16, tag="p")
                            nc.scalar.activation(
                                out=p, in_=sps[:, :NT], func=Exp, bias=0.0,
                                scale=SCALE,
                            )
                            pm = pp.tile([128, NT], BF16, tag="pm")
                            nc.vector.tensor_mul(
                                pm, p,
                                mask_t[(half, oc // 2)][:, oc % 2, ts(j, NT)],
                            )
                            pms[oc] = pm
                        if step >= LAG:
                            oc = step - LAG
                            pm = pms.pop(oc)
                            nc.tensor.matmul(
                                outp[j],
                                vp_sb[:, oc, h, :],
                                pm,
                                start=(oc == 0),
                                stop=(oc == OC - 1),
                            )
                            nc.tensor.matmul(
                                rp[32 * j : 32 * j + 1, :],
                                ones_col,
                                pm,
                                start=(oc == 0),
                                stop=(oc == OC - 1),
                            )
                            if j == 1:
                                for n, item in enumerate(inject_j1.get(oc, [])):
                                    outproj_item(*item, "facc", n % 2 == 1, 1)
                    nc.vector.tensor_copy(osb[:, ts(j, NT)], outp[j])
                    norm_chain(h, i0, rp, osb, (j,), rowoff=32)

            # half0 attention; its out-projection rides inside half1-h0
            # (items start at oc=4 so half0-h1's normalization chain has
            # finished; the last 4 ocs carry two items each).
            attention(0, 0, {})
            attention(0, 1, {}, prefetch=True)
            half0_items = [(it, dc) for it in (0, 1) for dc in range(DOUT // 128)]
            sched = {oc: [half0_items.pop(0)] for oc in range(4, 12)}
            for oc in range(12, 16):
                sched[oc] = [half0_items.pop(0), half0_items.pop(0)]
            attention(1, 0, sched)
            it2 = [(2, dc) for dc in range(DOUT // 128)]
            attention_jsplit(1, 1, {oc: [it2.pop(0)] for oc in range(8, 16)})
            # tail: it=3 out-projection, alternating DVE/ACT evacuation,
            # 3 PSUM tiles in flight (facc + the two freed s-tag slots).
            for n, dc in enumerate(range(DOUT // 128)):
                if n % 3 == 0:
                    outproj_item(3, dc, "facc", n % 2 == 1, 1)
                else:
                    outproj_item(3, dc, "s", n % 2 == 1, 2)

    return nc


_NC_CACHE = None


def _get_nc():
    global _NC_CACHE
    if _NC_CACHE is None:
        nc = build()
        nc.compile()
        _NC_CACHE = nc
    return _NC_CACHE


def make_in_maps(q, k, v, mask, Wq, bq, Wk, bk, Wv, bv, Wo, bo):
    bf = ml_dtypes.bfloat16
    q = np.asarray(q, np.float32)
    k = np.asarray(k, np.float32)
    v = np.asarray(v, np.float32)
    mask = np.asarray(mask)
    Ws = [np.asarray(w, np.float32) for w in (Wq, Wk, Wv)]
    bs = [np.asarray(b_, np.float32) for b_ in (bq, bk, bv)]
    Wo = np.asarray(Wo, np.float32)
    bo = np.asarray(bo, np.float32)

    xTb, maskTb = [], []
    for b in range(B):
        # [3, KP, 128, 2, S]: DRAM rows 256*j2 + 2p + i on partition p
        xTb.append(
            np.ascontiguousarray(
                np.stack([q[b].T, k[b].T, v[b].T]).reshape(3, KP, 128, 2, S)
            ).astype(bf)
        )
        # [2, OC2, 128, 2, 1024]: mask key rows 256*oc2 + 2p + i, column
        # half hf — matches the on-chip stride-2 key-chunk ordinals
        mT = np.ascontiguousarray(mask[b].T.astype(np.float32))
        maskTb.append(
            np.ascontiguousarray(
                np.stack(
                    [
                        mT[:, :2 * NT].reshape(OC2, 128, 2, 2 * NT),
                        mT[:, 2 * NT :].reshape(OC2, 128, 2, 2 * NT),
                    ]
                )
            ).astype(bf)
        )
    # weights with the x pair-row permutation absorbed:
    # w2[p, t, j2, i, h, d] = W_t[256*j2 + 2p + i, d*H + head]
    Wr = [W.reshape(KP, 128, 2, D, H) for W in Ws]
    br = [b_.reshape(D, H) for b_ in bs]

    ident = np.eye(128, dtype=np.float32).astype(bf)

    in_maps = []
    for c in range(8):
        b = c // 4
        h0 = NH * (c % 4)
        w_core = np.empty((128, 3, KP, 2, NH, D), np.float32)
        for t in range(3):
            for hi in range(NH):
                # Wr[t][j2, p, i, :, h0+hi] -> [p, j2, i, d]
                w_core[:, t, :, :, hi, :] = Wr[t][:, :, :, :, h0 + hi].transpose(
                    1, 0, 2, 3
                )
        b_core = np.empty((128, 3, NH), np.float32)
        for t in range(3):
            for hi in range(NH):
                b_core[:, t, hi] = br[t][:, h0 + hi]
        wo_core = np.stack([Wo[h0 + hi :: H, :] for hi in range(NH)], axis=1)
        bo_core = bo if c % 4 == 0 else np.zeros_like(bo)
        in_maps.append(
            {
                "xT": xTb[b],
                "ident": ident,
                "maskT": maskTb[b],
                "w_qkv": np.ascontiguousarray(w_core).astype(bf),
                "b_qkv": np.ascontiguousarray(b_core),
                "wo": np.ascontiguousarray(wo_core).astype(bf),
                "bo": np.ascontiguousarray(bo_core.reshape(DOUT // 128, 128).T),
            }
        )
    return in_maps


def unshard(results):
    out = np.zeros((B, DOUT, S), np.float32)
    for c in range(8):
        out[c // 4] += np.asarray(results[c]["outT"], np.float32)
    return np.ascontiguousarray(out.transpose(0, 2, 1))


def kernel(**inputs):
    in_maps = make_in_maps(**inputs)
    nc = _get_nc()
    res = run_bass_kernel_spmd(nc, in_maps, core_ids=list(range(8)))
    return unshard(res.results)
